# revision 26
# baseline (speedup 1.0000x reference)
"""Trainium2 Bass kernel for CrossAttention (B=4, T=2048, S=4096, D=256, H=8, Dh=32).

Sharding: 8 cores = 4 batches x 2 T-halves (each core owns 1024 query rows of
one batch, all heads). No collectives needed: each core computes its full
output rows; host concatenates.

Per-core dataflow (all "T"-like dims on the free axis, contractions on
partitions):
  xT [256, 1024], cT [256, 4096] via PE transposes (fp32 in, fp16 out)
  qT = w_q^T @ xT     [256(hid), 1024]
  kT = w_k^T @ cT     [256(hid), 4096]
  v  = cT^T @ w_v     [4096(S), 256] stored interleaved with a ones column
                      per head (v' [S, 33] per head) so attn@v' also yields
                      the softmax denominator for free.
  per (head, S-tile of 128):
    sT = kT_h_tile^T @ qT_h   [128(S), 1024(T)]  (fp16 matmul -> PSUM fp32)
    attnT = exp(sT * scale)   (ScalarE, PSUM->SBUF fp16)
    outT'_h += v'_h_tile^T @ attnT   [33, 1024] accumulated in PSUM fp32
  normalize outT by the broadcast reciprocal denominator,
  out = outT^T @ w_out + b_out.

Matmuls use fp16 operands (1 cycle/row; fp32 is 4x) with fp32 PSUM
accumulation. The structure is shaped by a hardware constraint: a PE
instruction can carry only ONE semaphore wait, so every matmul is arranged
to depend on at most one other engine (one shared PSUM pool, accumulator
dumps on ScalarE so slot releases merge with the exp waits, etc).
"""

import sys

if "/opt/trn_rl_repo" not in sys.path:
    sys.path.insert(0, "/opt/trn_rl_repo")

from contextlib import ExitStack

import numpy as np

import concourse.bass as bass
import concourse.tile as tile
from concourse import bacc
from concourse import mybir
from concourse.bass_utils import run_bass_kernel_spmd

B, T, S, D, H, Dh = 4, 2048, 4096, 256, 8, 32
TL = T // 2  # 1024 query rows per core
NXT = TL // 128  # 8 x tiles
SCALE = Dh ** -0.5
FP = mybir.dt.float32
F16 = mybir.dt.bfloat16
NST = S // 128  # 32 S-tiles
VW = H * (Dh + 1)  # 264 packed v' columns per S-tile
# head h -> (triple tile, 32-row block): heads grouped 3+3+2 so every row
# block starts at partition 0/32/64 (hardware base-partition constraint)
TRIP = [(h // 3, h % 3) for h in range(H)]
TRIP_HEADS = [[0, 1, 2], [3, 4, 5], [6, 7]]


def build_bass():
    nc = bacc.Bacc()
    ident_d = nc.declare_dram_parameter("ident", [128, 128], FP, isOutput=False)
    x_d = nc.declare_dram_parameter("x", [TL, D], FP, isOutput=False)
    ctx_d = nc.declare_dram_parameter("context", [S, D], FP, isOutput=False)
    wq_d = nc.declare_dram_parameter("w_q", [D, D], FP, isOutput=False)
    wkv_d = nc.declare_dram_parameter("w_kv", [D, 2 * D], FP, isOutput=False)
    wout_d = nc.declare_dram_parameter("w_out", [D, D], FP, isOutput=False)
    bout_d = nc.declare_dram_parameter("b_out", [1, D], FP, isOutput=False)
    out_d = nc.declare_dram_parameter("out", [TL, D], FP, isOutput=True)

    with tile.TileContext(nc) as tc, ExitStack() as ctx:
        consts = ctx.enter_context(tc.tile_pool(name="consts", bufs=1))
        persist = ctx.enter_context(tc.tile_pool(name="persist", bufs=1))
        psum = ctx.enter_context(tc.tile_pool(name="psum", bufs=2, space="PSUM"))
        attnp = ctx.enter_context(tc.tile_pool(name="attn", bufs=4))
        dnp = ctx.enter_context(tc.tile_pool(name="dnp", bufs=8))
        fstage = ctx.enter_context(tc.tile_pool(name="fstage", bufs=8))

        identity = consts.tile([128, 128], FP, tag="identity", name="identity")
        # hsel[b] [1, 96]: ones in columns 32b..32b+32 — builds the per-head
        # reciprocal broadcast via K=1 accumulating matmuls
        hsel = []
        for b in range(3):
            m = consts.tile([1, 96], F16, tag=f"hsel{b}", name=f"hsel{b}")
            nc.vector.memset(m, 0.0)
            nc.vector.memset(m[0:1, 32 * b : 32 * b + 32], 1.0)
            hsel.append(m)

        wq = [persist.tile([128, D], F16, tag=f"wq{j}", name=f"wq{j}") for j in range(2)]
        wkv = [persist.tile([128, 2 * D], F16, tag=f"wkv{j}", name=f"wkv{j}") for j in range(2)]
        wo_rows = [96, 96, 64]
        woutg = [
            persist.tile([wo_rows[t], D], F16, tag=f"woutg{t}", name=f"woutg{t}")
            for t in range(3)
        ]
        bias_b = persist.tile([128, D], FP, tag="bias_b", name="bias_b")
        bias_c = persist.tile([128, D], FP, tag="bias_c", name="bias_c")
        # 2 heads per tile (base-partition constraint)
        qT = [persist.tile([64, TL], F16, tag=f"qT{j}", name=f"qT{j}") for j in range(4)]
        kT = [persist.tile([64, S], F16, tag=f"kT{j}", name=f"kT{j}") for j in range(4)]
        vP = persist.tile([128, NST * VW], F16, tag="vP", name="vP")
        dumpT = [
            persist.tile([96, TL], F16, tag=f"dumpT{t}", name=f"dumpT{t}")
            for t in range(3)
        ]
        outTh = [
            persist.tile([96, TL], F16, tag=f"outTh{t}", name=f"outTh{t}")
            for t in range(3)
        ]
        rcp = [persist.tile([96, TL], F16, tag=f"rcp{t}", name=f"rcp{t}") for t in range(3)]
        x_all = persist.tile([128, NXT, D], FP, tag="x_all", name="x_all")
        c_all = persist.tile([128, NST, D], FP, tag="c_all", name="c_all")
        xT = [persist.tile([128, TL], F16, tag=f"xT{j}", name=f"xT{j}") for j in range(2)]
        cT = [persist.tile([128, S], F16, tag=f"cT{j}", name=f"cT{j}") for j in range(2)]
        wstage = [
            persist.tile([128, 3 * D], FP, tag=f"wstage{j}", name=f"wstage{j}")
            for j in range(2)
        ]
        wso = [
            persist.tile([wo_rows[t], D], FP, tag=f"wso{t}", name=f"wso{t}")
            for t in range(3)
        ]

        # ---- Phase 0: loads + fp16 weight conversion ----
        # DMA issue order is load-bearing: the HW DGE queue is assigned
        # round-robin (mod 8) over DMA program order. ident is DMA #0 and
        # x_all #8 (same queue), c_all #9 -> the two first-touch transposes
        # each carry exactly one queue wait, and PE never needs a second one.
        nc.sync.dma_start(out=identity, in_=ident_d[:, :])
        for j in range(2):
            nc.sync.dma_start(out=wstage[j][:, 0:D], in_=wq_d[128 * j : 128 * j + 128, :])
            nc.sync.dma_start(
                out=wstage[j][:, D : 3 * D], in_=wkv_d[128 * j : 128 * j + 128, :]
            )
            nc.vector.tensor_copy(wq[j], wstage[j][:, 0:D])
            nc.vector.tensor_copy(wkv[j], wstage[j][:, D : 3 * D])
        ro = 0
        for t in range(3):
            nc.sync.dma_start(out=wso[t], in_=wout_d[ro : ro + wo_rows[t], :])
            nc.vector.tensor_copy(woutg[t], wso[t])
            ro += wo_rows[t]
        nc.sync.dma_start(out=x_all, in_=x_d.rearrange("(t p) d -> p t d", p=128))
        nc.sync.dma_start(out=c_all, in_=ctx_d.rearrange("(t p) d -> p t d", p=128))
        nc.sync.dma_start(out=bias_b, in_=bout_d[0:1, :].partition_broadcast(128))
        nc.vector.tensor_copy(bias_c, bias_b)

        # ---- Phase 1: transpose x and context ----
        def do_transpose(src_all, st, j, dstT):
            pt = psum.tile([128, 128], FP, tag="sc", name="pt")
            nc.tensor.transpose(pt, src_all[:, st, 128 * j : 128 * j + 128], identity)
            nc.vector.tensor_copy(dstT[:, 128 * st : 128 * st + 128], pt)

        # first touch of each DMA'd tensor rides its own fresh wait
        do_transpose(x_all, 0, 0, xT[0])
        do_transpose(c_all, 0, 0, cT[0])
        do_transpose(x_all, 0, 1, xT[1])
        do_transpose(c_all, 0, 1, cT[1])
        for t in range(1, NXT):
            for j in range(2):
                do_transpose(x_all, t, j, xT[j])
        for st in range(1, NST):
            for j in range(2):
                do_transpose(c_all, st, j, cT[j])

        # ---- Phase 2: projections ----
        for mj in range(2):
            for nt in range(TL // 512):
                pq = psum.tile([128, 512], FP, tag="sc", name="pq")
                for kj in range(2):
                    nc.tensor.matmul(
                        pq,
                        lhsT=wq[kj][:, 128 * mj : 128 * mj + 128],
                        rhs=xT[kj][:, 512 * nt : 512 * nt + 512],
                        start=(kj == 0),
                        stop=(kj == 1),
                    )
                for half in range(2):
                    nc.vector.tensor_copy(
                        qT[2 * mj + half][:, 512 * nt : 512 * nt + 512],
                        pq[64 * half : 64 * half + 64, :],
                    )
        for mj in range(2):
            for nt in range(S // 512):
                pk = psum.tile([128, 512], FP, tag="sc", name="pk")
                for kj in range(2):
                    nc.tensor.matmul(
                        pk,
                        lhsT=wkv[kj][:, 128 * mj : 128 * mj + 128],
                        rhs=cT[kj][:, 512 * nt : 512 * nt + 512],
                        start=(kj == 0),
                        stop=(kj == 1),
                    )
                for half in range(2):
                    nc.vector.tensor_copy(
                        kT[2 * mj + half][:, 512 * nt : 512 * nt + 512],
                        pk[64 * half : 64 * half + 64, :],
                    )
        for st in range(NST):
            pv = psum.tile([128, D], FP, tag="sc", name="pv")
            for kj in range(2):
                nc.tensor.matmul(
                    pv,
                    lhsT=cT[kj][:, 128 * st : 128 * st + 128],
                    rhs=wkv[kj][:, D : 2 * D],
                    start=(kj == 0),
                    stop=(kj == 1),
                )
            dst = vP[:, VW * st : VW * st + VW].rearrange("p (h w) -> p h w", h=H)[
                :, :, 0:Dh
            ]
            nc.vector.tensor_copy(dst, pv.rearrange("p (h w) -> p h w", h=H))
        ones_cols = vP.rearrange("p (s h w) -> p s h w", s=NST, h=H)[:, :, :, Dh : Dh + 1]
        nc.vector.memset(ones_cols, 1.0)

        # ---- Phase 3: fused attention ----
        dn_tiles = []
        for h in range(H):
            jj, aa = h // 2, h % 2
            tt_, bb_ = TRIP[h]
            acc = psum.tile([Dh + 1, TL], FP, tag="acc", name="acc")
            for st in range(NST):
                sc = psum.tile([128, TL], FP, tag="sc", name="sc")
                for nt in range(2):
                    nc.tensor.matmul(
                        sc[:, 512 * nt : 512 * nt + 512],
                        lhsT=kT[jj][32 * aa : 32 * aa + 32, 128 * st : 128 * st + 128],
                        rhs=qT[jj][32 * aa : 32 * aa + 32, 512 * nt : 512 * nt + 512],
                        start=True,
                        stop=True,
                        skip_group_check=True,
                    )
                at = attnp.tile([128, TL], F16, tag="at", name="at")
                nc.scalar.activation(
                    at, sc, mybir.ActivationFunctionType.Exp, scale=SCALE
                )
                for nt in range(2):
                    nc.tensor.matmul(
                        acc[:, 512 * nt : 512 * nt + 512],
                        lhsT=vP[:, VW * st + (Dh + 1) * h : VW * st + (Dh + 1) * h + Dh + 1],
                        rhs=at[:, 512 * nt : 512 * nt + 512],
                        start=(st == 0),
                        stop=(st == NST - 1),
                        skip_group_check=True,
                    )
            # dump accumulator on ScalarE: the PSUM slot release then shares
            # the ACT semaphore with the exp waits (single-wait rule)
            nc.scalar.copy(dumpT[tt_][32 * bb_ : 32 * bb_ + 32, :], acc[0:Dh, :])
            dnt = dnp.tile([1, TL], F16, tag="dn", name="dn")
            nc.scalar.copy(dnt, acc[Dh : Dh + 1, :])
            dn_tiles.append(dnt)

        # ---- Phase 4: normalize + output projection ----
        for t in range(3):
            rp = psum.tile([96, TL], FP, tag="sc", name="rp")
            heads = TRIP_HEADS[t]
            for bi, h in enumerate(heads):
                for nt in range(2):
                    nc.tensor.matmul(
                        rp[:, 512 * nt : 512 * nt + 512],
                        lhsT=hsel[bi],
                        rhs=dn_tiles[h][:, 512 * nt : 512 * nt + 512],
                        start=(bi == 0),
                        stop=(bi == len(heads) - 1),
                        skip_group_check=True,
                    )
            # wide reciprocal of the broadcast denominators (a [1, TL]
            # single-partition reciprocal runs on one DVE lane ~50x slower)
            rps = rcp[t]
            with nc.allow_low_precision("softmax denominators are well-conditioned"):
                nc.vector.reciprocal(rps, rp)
            for bi in range(len(heads)):
                nc.vector.tensor_mul(
                    outTh[t][32 * bi : 32 * bi + 32, :],
                    dumpT[t][32 * bi : 32 * bi + 32, :],
                    rps[32 * bi : 32 * bi + 32, :],
                )
        for tt in range(TL // 128):
            fin = psum.tile([128, D], FP, tag="sc", name="fin")
            for t in range(3):
                nc.tensor.matmul(
                    fin,
                    lhsT=outTh[t][0 : wo_rows[t], 128 * tt : 128 * tt + 128],
                    rhs=woutg[t],
                    start=(t == 0),
                    stop=(t == 2),
                )
            outs = fstage.tile([128, D], FP, tag="outs", name="outs")
            nc.vector.tensor_add(outs, fin, bias_c)
            nc.sync.dma_start(out=out_d[128 * tt : 128 * tt + 128, :], in_=outs)

    nc.compile()
    return nc


_NC = None


def kernel(**inputs):
    global _NC
    x = np.ascontiguousarray(inputs["x"], dtype=np.float32)
    context = np.ascontiguousarray(inputs["context"], dtype=np.float32)
    w_q = np.ascontiguousarray(inputs["w_q"], dtype=np.float32)
    w_kv = np.ascontiguousarray(inputs["w_kv"], dtype=np.float32)
    w_out = np.ascontiguousarray(inputs["w_out"], dtype=np.float32)
    b_out = np.ascontiguousarray(inputs["b_out"], dtype=np.float32).reshape(1, D)

    if _NC is None:
        _NC = build_bass()
    nc = _NC

    in_maps = []
    for c in range(8):
        b, half = c // 2, c % 2
        in_maps.append(
            {
                "ident": np.eye(128, dtype=np.float32),
                "x": np.ascontiguousarray(x[b, TL * half : TL * half + TL, :]),
                "context": np.ascontiguousarray(context[b]),
                "w_q": w_q,
                "w_kv": w_kv,
                "w_out": w_out,
                "b_out": b_out,
            }
        )
    res = run_bass_kernel_spmd(nc, in_maps, core_ids=list(range(8)))
    out = np.empty((B, T, D), dtype=np.float32)
    for c in range(8):
        b, half = c // 2, c % 2
        out[b, TL * half : TL * half + TL, :] = res.results[c]["out"]
    return out


if __name__ == "__main__":
    rng = np.random.default_rng(0)
    ins = {
        "x": rng.standard_normal((B, T, D), dtype=np.float32),
        "context": rng.standard_normal((B, S, D), dtype=np.float32),
        "w_q": rng.standard_normal((D, D), dtype=np.float32) * D**-0.5,
        "w_kv": rng.standard_normal((D, 2 * D), dtype=np.float32) * D**-0.5,
        "w_out": rng.standard_normal((D, D), dtype=np.float32) * D**-0.5,
        "b_out": rng.standard_normal((D,), dtype=np.float32) * 0.01,
    }
    out = kernel(**ins)
    print(out.shape, out.dtype, np.abs(out).mean())


# revision 28
# speedup vs baseline: 1.0164x; 1.0164x over previous
"""Trainium2 Bass kernel for CrossAttention (B=4, T=2048, S=4096, D=256, H=8, Dh=32).

Sharding: 8 cores = 4 batches x 2 T-halves (each core owns 1024 query rows of
one batch, all heads). No collectives needed: each core computes its full
output rows; host concatenates.

Per-core dataflow (all "T"-like dims on the free axis, contractions on
partitions):
  xT [256, 1024], cT [256, 4096] via PE transposes (fp32 in, fp16 out)
  qT = w_q^T @ xT     [256(hid), 1024]
  kT = w_k^T @ cT     [256(hid), 4096]
  v  = cT^T @ w_v     [4096(S), 256] stored interleaved with a ones column
                      per head (v' [S, 33] per head) so attn@v' also yields
                      the softmax denominator for free.
  per (head, S-tile of 128):
    sT = kT_h_tile^T @ qT_h   [128(S), 1024(T)]  (fp16 matmul -> PSUM fp32)
    attnT = exp(sT * scale)   (ScalarE, PSUM->SBUF fp16)
    outT'_h += v'_h_tile^T @ attnT   [33, 1024] accumulated in PSUM fp32
  normalize outT by the broadcast reciprocal denominator,
  out = outT^T @ w_out + b_out.

Matmuls use fp16 operands (1 cycle/row; fp32 is 4x) with fp32 PSUM
accumulation. The structure is shaped by a hardware constraint: a PE
instruction can carry only ONE semaphore wait, so every matmul is arranged
to depend on at most one other engine (one shared PSUM pool, accumulator
dumps on ScalarE so slot releases merge with the exp waits, etc).
"""

import sys

if "/opt/trn_rl_repo" not in sys.path:
    sys.path.insert(0, "/opt/trn_rl_repo")

from contextlib import ExitStack

import numpy as np

import concourse.bass as bass
import concourse.tile as tile
from concourse import bacc
from concourse import mybir
from concourse.bass_utils import run_bass_kernel_spmd

B, T, S, D, H, Dh = 4, 2048, 4096, 256, 8, 32
TL = T // 2  # 1024 query rows per core
NXT = TL // 128  # 8 x tiles
SCALE = Dh ** -0.5
FP = mybir.dt.float32
F16 = mybir.dt.float16
NST = S // 128  # 32 S-tiles
VW = H * (Dh + 1)  # 264 packed v' columns per S-tile
# head h -> (triple tile, 32-row block): heads grouped 3+3+2 so every row
# block starts at partition 0/32/64 (hardware base-partition constraint)
TRIP = [(h // 3, h % 3) for h in range(H)]
TRIP_HEADS = [[0, 1, 2], [3, 4, 5], [6, 7]]


def build_bass():
    nc = bacc.Bacc()
    ident_d = nc.declare_dram_parameter("ident", [128, 128], FP, isOutput=False)
    x_d = nc.declare_dram_parameter("x", [TL, D], FP, isOutput=False)
    ctx_d = nc.declare_dram_parameter("context", [S, D], FP, isOutput=False)
    wq_d = nc.declare_dram_parameter("w_q", [D, D], FP, isOutput=False)
    wkv_d = nc.declare_dram_parameter("w_kv", [D, 2 * D], FP, isOutput=False)
    wout_d = nc.declare_dram_parameter("w_out", [D, D], FP, isOutput=False)
    bout_d = nc.declare_dram_parameter("b_out", [1, D], FP, isOutput=False)
    out_d = nc.declare_dram_parameter("out", [TL, D], FP, isOutput=True)

    with tile.TileContext(nc) as tc, ExitStack() as ctx:
        consts = ctx.enter_context(tc.tile_pool(name="consts", bufs=1))
        persist = ctx.enter_context(tc.tile_pool(name="persist", bufs=1))
        psum = ctx.enter_context(tc.tile_pool(name="psum", bufs=2, space="PSUM"))
        attnp = ctx.enter_context(tc.tile_pool(name="attn", bufs=4))
        dnp = ctx.enter_context(tc.tile_pool(name="dnp", bufs=8))
        fstage = ctx.enter_context(tc.tile_pool(name="fstage", bufs=8))

        identity = consts.tile([128, 128], FP, tag="identity", name="identity")
        # hsel[b] [1, 96]: ones in columns 32b..32b+32 — builds the per-head
        # reciprocal broadcast via K=1 accumulating matmuls
        hsel = []
        for b in range(3):
            m = consts.tile([1, 96], F16, tag=f"hsel{b}", name=f"hsel{b}")
            nc.vector.memset(m, 0.0)
            nc.vector.memset(m[0:1, 32 * b : 32 * b + 32], 1.0)
            hsel.append(m)

        wq = [persist.tile([128, D], F16, tag=f"wq{j}", name=f"wq{j}") for j in range(2)]
        wkv = [persist.tile([128, 2 * D], F16, tag=f"wkv{j}", name=f"wkv{j}") for j in range(2)]
        wo_rows = [96, 96, 64]
        woutg = [
            persist.tile([wo_rows[t], D], F16, tag=f"woutg{t}", name=f"woutg{t}")
            for t in range(3)
        ]
        bias_b = persist.tile([128, D], FP, tag="bias_b", name="bias_b")
        bias_c = persist.tile([128, D], FP, tag="bias_c", name="bias_c")
        # 2 heads per tile (base-partition constraint)
        qT = [persist.tile([64, TL], F16, tag=f"qT{j}", name=f"qT{j}") for j in range(4)]
        kT = [persist.tile([64, S], F16, tag=f"kT{j}", name=f"kT{j}") for j in range(4)]
        vP = persist.tile([128, NST * VW], F16, tag="vP", name="vP")
        dumpT = [
            persist.tile([96, TL], F16, tag=f"dumpT{t}", name=f"dumpT{t}")
            for t in range(3)
        ]
        outTh = [
            persist.tile([96, TL], F16, tag=f"outTh{t}", name=f"outTh{t}")
            for t in range(3)
        ]
        rcp = [persist.tile([96, TL], F16, tag=f"rcp{t}", name=f"rcp{t}") for t in range(3)]
        x_all = persist.tile([128, NXT, D], FP, tag="x_all", name="x_all")
        c_all = persist.tile([128, NST, D], FP, tag="c_all", name="c_all")
        xT = [persist.tile([128, TL], F16, tag=f"xT{j}", name=f"xT{j}") for j in range(2)]
        cT = [persist.tile([128, S], F16, tag=f"cT{j}", name=f"cT{j}") for j in range(2)]
        wstage = [
            persist.tile([128, 3 * D], FP, tag=f"wstage{j}", name=f"wstage{j}")
            for j in range(2)
        ]
        wso = [
            persist.tile([wo_rows[t], D], FP, tag=f"wso{t}", name=f"wso{t}")
            for t in range(3)
        ]

        # ---- Phase 0: loads + fp16 weight conversion ----
        # DMA issue order is load-bearing: the HW DGE queue is assigned
        # round-robin (mod 8) over DMA program order. ident is DMA #0 and
        # x_all #8 (same queue), c_all #9 -> the two first-touch transposes
        # each carry exactly one queue wait, and PE never needs a second one.
        nc.sync.dma_start(out=identity, in_=ident_d[:, :])
        for j in range(2):
            nc.sync.dma_start(out=wstage[j][:, 0:D], in_=wq_d[128 * j : 128 * j + 128, :])
            nc.sync.dma_start(
                out=wstage[j][:, D : 3 * D], in_=wkv_d[128 * j : 128 * j + 128, :]
            )
            nc.vector.tensor_copy(wq[j], wstage[j][:, 0:D])
            nc.vector.tensor_copy(wkv[j], wstage[j][:, D : 3 * D])
        ro = 0
        for t in range(3):
            nc.sync.dma_start(out=wso[t], in_=wout_d[ro : ro + wo_rows[t], :])
            nc.vector.tensor_copy(woutg[t], wso[t])
            ro += wo_rows[t]
        nc.sync.dma_start(out=x_all, in_=x_d.rearrange("(t p) d -> p t d", p=128))
        nc.sync.dma_start(out=c_all, in_=ctx_d.rearrange("(t p) d -> p t d", p=128))
        nc.sync.dma_start(out=bias_b, in_=bout_d[0:1, :].partition_broadcast(128))
        nc.vector.tensor_copy(bias_c, bias_b)

        # ---- Phase 1: transpose x and context ----
        # convert to fp16 first: fp32 PE transposes run 4 cycles/row
        xh = persist.tile([128, NXT, D], F16, tag="xh", name="xh")
        ch = persist.tile([128, NST, D], F16, tag="ch", name="ch")
        idh = consts.tile([128, 128], F16, tag="idh", name="idh")
        nc.vector.tensor_copy(idh, identity)
        nc.vector.tensor_copy(xh, x_all)
        nc.vector.tensor_copy(ch, c_all)

        def do_transpose(src_all, st, j, dstT):
            pt = psum.tile([128, 128], F16, tag="sc", name="pt")
            nc.tensor.transpose(pt, src_all[:, st, 128 * j : 128 * j + 128], idh)
            nc.vector.tensor_copy(dstT[:, 128 * st : 128 * st + 128], pt)

        for t in range(NXT):
            for j in range(2):
                do_transpose(xh, t, j, xT[j])
        for st in range(NST):
            for j in range(2):
                do_transpose(ch, st, j, cT[j])

        # ---- Phase 2: projections ----
        for mj in range(2):
            for nt in range(TL // 512):
                pq = psum.tile([128, 512], FP, tag="sc", name="pq")
                for kj in range(2):
                    nc.tensor.matmul(
                        pq,
                        lhsT=wq[kj][:, 128 * mj : 128 * mj + 128],
                        rhs=xT[kj][:, 512 * nt : 512 * nt + 512],
                        start=(kj == 0),
                        stop=(kj == 1),
                    )
                for half in range(2):
                    nc.vector.tensor_copy(
                        qT[2 * mj + half][:, 512 * nt : 512 * nt + 512],
                        pq[64 * half : 64 * half + 64, :],
                    )
        for mj in range(2):
            for nt in range(S // 512):
                pk = psum.tile([128, 512], FP, tag="sc", name="pk")
                for kj in range(2):
                    nc.tensor.matmul(
                        pk,
                        lhsT=wkv[kj][:, 128 * mj : 128 * mj + 128],
                        rhs=cT[kj][:, 512 * nt : 512 * nt + 512],
                        start=(kj == 0),
                        stop=(kj == 1),
                    )
                for half in range(2):
                    nc.vector.tensor_copy(
                        kT[2 * mj + half][:, 512 * nt : 512 * nt + 512],
                        pk[64 * half : 64 * half + 64, :],
                    )
        for st in range(NST):
            pv = psum.tile([128, D], FP, tag="sc", name="pv")
            for kj in range(2):
                nc.tensor.matmul(
                    pv,
                    lhsT=cT[kj][:, 128 * st : 128 * st + 128],
                    rhs=wkv[kj][:, D : 2 * D],
                    start=(kj == 0),
                    stop=(kj == 1),
                )
            dst = vP[:, VW * st : VW * st + VW].rearrange("p (h w) -> p h w", h=H)[
                :, :, 0:Dh
            ]
            nc.vector.tensor_copy(dst, pv.rearrange("p (h w) -> p h w", h=H))
        ones_cols = vP.rearrange("p (s h w) -> p s h w", s=NST, h=H)[:, :, :, Dh : Dh + 1]
        nc.vector.memset(ones_cols, 1.0)

        # ---- Phase 3: fused attention ----
        # S-tiles processed in pairs: grouping the K=32 score matmuls and the
        # K=128 attn@v matmuls into runs halves the PE K-geometry switches
        # (~200ns each)
        dn_tiles = []
        for h in range(H):
            jj, aa = h // 2, h % 2
            tt_, bb_ = TRIP[h]
            acc = psum.tile([Dh + 1, TL], FP, tag="acc", name="acc")
            for sp in range(NST // 2):
                scs = []
                for st in (2 * sp, 2 * sp + 1):
                    sc = psum.tile([128, TL], FP, tag="sc", name="sc")
                    for nt in range(2):
                        nc.tensor.matmul(
                            sc[:, 512 * nt : 512 * nt + 512],
                            lhsT=kT[jj][32 * aa : 32 * aa + 32, 128 * st : 128 * st + 128],
                            rhs=qT[jj][32 * aa : 32 * aa + 32, 512 * nt : 512 * nt + 512],
                            start=True,
                            stop=True,
                            skip_group_check=True,
                        )
                    scs.append(sc)
                ats = []
                for sc in scs:
                    at = attnp.tile([128, TL], F16, tag="at", name="at")
                    nc.scalar.activation(
                        at, sc, mybir.ActivationFunctionType.Exp, scale=SCALE
                    )
                    ats.append(at)
                for i, st in enumerate((2 * sp, 2 * sp + 1)):
                    at = ats[i]
                    for nt in range(2):
                        nc.tensor.matmul(
                            acc[:, 512 * nt : 512 * nt + 512],
                            lhsT=vP[:, VW * st + (Dh + 1) * h : VW * st + (Dh + 1) * h + Dh + 1],
                            rhs=at[:, 512 * nt : 512 * nt + 512],
                            start=(st == 0),
                            stop=(st == NST - 1),
                            skip_group_check=True,
                        )
            nc.vector.tensor_copy(dumpT[tt_][32 * bb_ : 32 * bb_ + 32, :], acc[0:Dh, :])
            dnt = dnp.tile([1, TL], F16, tag="dn", name="dn")
            nc.vector.tensor_copy(dnt, acc[Dh : Dh + 1, :])
            dn_tiles.append(dnt)

        # ---- Phase 4: normalize + output projection ----
        for t in range(3):
            rp = psum.tile([96, TL], FP, tag="sc", name="rp")
            heads = TRIP_HEADS[t]
            for bi, h in enumerate(heads):
                for nt in range(2):
                    nc.tensor.matmul(
                        rp[:, 512 * nt : 512 * nt + 512],
                        lhsT=hsel[bi],
                        rhs=dn_tiles[h][:, 512 * nt : 512 * nt + 512],
                        start=(bi == 0),
                        stop=(bi == len(heads) - 1),
                        skip_group_check=True,
                    )
            # wide reciprocal of the broadcast denominators (a [1, TL]
            # single-partition reciprocal runs on one DVE lane ~50x slower)
            rps = rcp[t]
            with nc.allow_low_precision("softmax denominators are well-conditioned"):
                nc.vector.reciprocal(rps, rp)
            for bi in range(len(heads)):
                nc.vector.tensor_mul(
                    outTh[t][32 * bi : 32 * bi + 32, :],
                    dumpT[t][32 * bi : 32 * bi + 32, :],
                    rps[32 * bi : 32 * bi + 32, :],
                )
        for tt in range(TL // 128):
            fin = psum.tile([128, D], FP, tag="sc", name="fin")
            for t in range(3):
                nc.tensor.matmul(
                    fin,
                    lhsT=outTh[t][0 : wo_rows[t], 128 * tt : 128 * tt + 128],
                    rhs=woutg[t],
                    start=(t == 0),
                    stop=(t == 2),
                )
            outs = fstage.tile([128, D], FP, tag="outs", name="outs")
            nc.vector.tensor_add(outs, fin, bias_c)
            nc.sync.dma_start(out=out_d[128 * tt : 128 * tt + 128, :], in_=outs)

    nc.compile()
    return nc


_NC = None


def kernel(**inputs):
    global _NC
    x = np.ascontiguousarray(inputs["x"], dtype=np.float32)
    context = np.ascontiguousarray(inputs["context"], dtype=np.float32)
    w_q = np.ascontiguousarray(inputs["w_q"], dtype=np.float32)
    w_kv = np.ascontiguousarray(inputs["w_kv"], dtype=np.float32)
    w_out = np.ascontiguousarray(inputs["w_out"], dtype=np.float32)
    b_out = np.ascontiguousarray(inputs["b_out"], dtype=np.float32).reshape(1, D)

    if _NC is None:
        _NC = build_bass()
    nc = _NC

    in_maps = []
    for c in range(8):
        b, half = c // 2, c % 2
        in_maps.append(
            {
                "ident": np.eye(128, dtype=np.float32),
                "x": np.ascontiguousarray(x[b, TL * half : TL * half + TL, :]),
                "context": np.ascontiguousarray(context[b]),
                "w_q": w_q,
                "w_kv": w_kv,
                "w_out": w_out,
                "b_out": b_out,
            }
        )
    res = run_bass_kernel_spmd(nc, in_maps, core_ids=list(range(8)))
    out = np.empty((B, T, D), dtype=np.float32)
    for c in range(8):
        b, half = c // 2, c % 2
        out[b, TL * half : TL * half + TL, :] = res.results[c]["out"]
    return out


if __name__ == "__main__":
    rng = np.random.default_rng(0)
    ins = {
        "x": rng.standard_normal((B, T, D), dtype=np.float32),
        "context": rng.standard_normal((B, S, D), dtype=np.float32),
        "w_q": rng.standard_normal((D, D), dtype=np.float32) * D**-0.5,
        "w_kv": rng.standard_normal((D, 2 * D), dtype=np.float32) * D**-0.5,
        "w_out": rng.standard_normal((D, D), dtype=np.float32) * D**-0.5,
        "b_out": rng.standard_normal((D,), dtype=np.float32) * 0.01,
    }
    out = kernel(**ins)
    print(out.shape, out.dtype, np.abs(out).mean())


# revision 30
# speedup vs baseline: 1.5227x; 1.4982x over previous
"""Trainium2 Bass kernel for CrossAttention (B=4, T=2048, S=4096, D=256, H=8, Dh=32).

Sharding: 8 cores = 4 batches x 2 T-halves (each core owns 1024 query rows of
one batch, all heads). No collectives needed: each core computes its full
output rows; host concatenates.

Per-core dataflow (all "T"-like dims on the free axis, contractions on
partitions):
  xT [256, 1024], cT [256, 4096] via PE transposes (fp32 in, fp16 out)
  qT = w_q^T @ xT     [256(hid), 1024]
  kT = w_k^T @ cT     [256(hid), 4096]
  v  = cT^T @ w_v     [4096(S), 256] stored interleaved with a ones column
                      per head (v' [S, 33] per head) so attn@v' also yields
                      the softmax denominator for free.
  per (head, S-tile of 128):
    sT = kT_h_tile^T @ qT_h   [128(S), 1024(T)]  (fp16 matmul -> PSUM fp32)
    attnT = exp(sT * scale)   (ScalarE, PSUM->SBUF fp16)
    outT'_h += v'_h_tile^T @ attnT   [33, 1024] accumulated in PSUM fp32
  normalize outT by the broadcast reciprocal denominator,
  out = outT^T @ w_out + b_out.

Matmuls use fp16 operands (1 cycle/row; fp32 is 4x) with fp32 PSUM
accumulation. The structure is shaped by a hardware constraint: a PE
instruction can carry only ONE semaphore wait, so every matmul is arranged
to depend on at most one other engine (one shared PSUM pool, accumulator
dumps on ScalarE so slot releases merge with the exp waits, etc).
"""

import sys

if "/opt/trn_rl_repo" not in sys.path:
    sys.path.insert(0, "/opt/trn_rl_repo")

from contextlib import ExitStack

import numpy as np

import concourse.bass as bass
import concourse.tile as tile
from concourse import bacc
from concourse import mybir
from concourse.bass_utils import run_bass_kernel_spmd

B, T, S, D, H, Dh = 4, 2048, 4096, 256, 8, 32
TL = T // 2  # 1024 query rows per core
NXT = TL // 128  # 8 x tiles
SCALE = Dh ** -0.5
FP = mybir.dt.float32
F16 = mybir.dt.float16
NST = S // 128  # 32 S-tiles
VW = H * (Dh + 1)  # 264 packed v' columns per S-tile
# head h -> (triple tile, 32-row block): heads grouped 3+3+2 so every row
# block starts at partition 0/32/64 (hardware base-partition constraint)
TRIP = [(h // 3, h % 3) for h in range(H)]
TRIP_HEADS = [[0, 1, 2], [3, 4, 5], [6, 7]]


def build_bass():
    nc = bacc.Bacc()
    ident_d = nc.declare_dram_parameter("ident", [128, 128], FP, isOutput=False)
    x_d = nc.declare_dram_parameter("x", [TL, D], FP, isOutput=False)
    ctx_d = nc.declare_dram_parameter("context", [S, D], FP, isOutput=False)
    wq_d = nc.declare_dram_parameter("w_q", [D, D], FP, isOutput=False)
    wkv_d = nc.declare_dram_parameter("w_kv", [D, 2 * D], FP, isOutput=False)
    wout_d = nc.declare_dram_parameter("w_out", [D, D], FP, isOutput=False)
    bout_d = nc.declare_dram_parameter("b_out", [1, D], FP, isOutput=False)
    out_d = nc.declare_dram_parameter("out", [TL, D], FP, isOutput=True)

    with tile.TileContext(nc) as tc, ExitStack() as ctx:
        consts = ctx.enter_context(tc.tile_pool(name="consts", bufs=1))
        persist = ctx.enter_context(tc.tile_pool(name="persist", bufs=1))
        psum = ctx.enter_context(tc.tile_pool(name="psum", bufs=3, space="PSUM"))
        attnp = ctx.enter_context(tc.tile_pool(name="attn", bufs=4))
        dnp = ctx.enter_context(tc.tile_pool(name="dnp", bufs=8))
        fstage = ctx.enter_context(tc.tile_pool(name="fstage", bufs=8))

        identity = consts.tile([128, 128], FP, tag="identity", name="identity")
        # hsel[b] [1, 96]: ones in columns 32b..32b+32 — builds the per-head
        # reciprocal broadcast via K=1 accumulating matmuls
        hsel = []
        for b in range(3):
            m = consts.tile([1, 96], F16, tag=f"hsel{b}", name=f"hsel{b}")
            nc.vector.memset(m, 0.0)
            nc.vector.memset(m[0:1, 32 * b : 32 * b + 32], 1.0)
            hsel.append(m)

        wq = [persist.tile([128, D], F16, tag=f"wq{j}", name=f"wq{j}") for j in range(2)]
        wkv = [persist.tile([128, 2 * D], F16, tag=f"wkv{j}", name=f"wkv{j}") for j in range(2)]
        wo_rows = [96, 96, 64]
        woutg = [
            persist.tile([wo_rows[t], D], F16, tag=f"woutg{t}", name=f"woutg{t}")
            for t in range(3)
        ]
        bias_b = persist.tile([128, D], FP, tag="bias_b", name="bias_b")
        bias_c = persist.tile([128, D], FP, tag="bias_c", name="bias_c")
        # 2 heads per tile (base-partition constraint)
        qT = [persist.tile([64, TL], F16, tag=f"qT{j}", name=f"qT{j}") for j in range(4)]
        kT = [persist.tile([64, S], F16, tag=f"kT{j}", name=f"kT{j}") for j in range(4)]
        vP = persist.tile([128, NST * VW], F16, tag="vP", name="vP")
        dumpT = [
            persist.tile([96, TL], F16, tag=f"dumpT{t}", name=f"dumpT{t}")
            for t in range(3)
        ]
        outTh = [
            persist.tile([96, TL], F16, tag=f"outTh{t}", name=f"outTh{t}")
            for t in range(3)
        ]
        rcp = [persist.tile([96, TL], F16, tag=f"rcp{t}", name=f"rcp{t}") for t in range(3)]
        x_all = persist.tile([128, NXT, D], FP, tag="x_all", name="x_all")
        c_all = persist.tile([128, NST, D], FP, tag="c_all", name="c_all")
        xT = [persist.tile([128, TL], F16, tag=f"xT{j}", name=f"xT{j}") for j in range(2)]
        cT = [persist.tile([128, S], F16, tag=f"cT{j}", name=f"cT{j}") for j in range(2)]
        wstage = [
            persist.tile([128, 3 * D], FP, tag=f"wstage{j}", name=f"wstage{j}")
            for j in range(2)
        ]
        wso = [
            persist.tile([wo_rows[t], D], FP, tag=f"wso{t}", name=f"wso{t}")
            for t in range(3)
        ]

        # ---- Phase 0: loads + fp16 weight conversion ----
        # DMA issue order is load-bearing: the HW DGE queue is assigned
        # round-robin (mod 8) over DMA program order. ident is DMA #0 and
        # x_all #8 (same queue), c_all #9 -> the two first-touch transposes
        # each carry exactly one queue wait, and PE never needs a second one.
        nc.sync.dma_start(out=identity, in_=ident_d[:, :])
        for j in range(2):
            nc.sync.dma_start(out=wstage[j][:, 0:D], in_=wq_d[128 * j : 128 * j + 128, :])
            nc.sync.dma_start(
                out=wstage[j][:, D : 3 * D], in_=wkv_d[128 * j : 128 * j + 128, :]
            )
            nc.vector.tensor_copy(wq[j], wstage[j][:, 0:D])
            nc.vector.tensor_copy(wkv[j], wstage[j][:, D : 3 * D])
        ro = 0
        for t in range(3):
            nc.sync.dma_start(out=wso[t], in_=wout_d[ro : ro + wo_rows[t], :])
            nc.vector.tensor_copy(woutg[t], wso[t])
            ro += wo_rows[t]
        nc.sync.dma_start(out=x_all, in_=x_d.rearrange("(t p) d -> p t d", p=128))
        nc.sync.dma_start(out=c_all, in_=ctx_d.rearrange("(t p) d -> p t d", p=128))
        nc.sync.dma_start(out=bias_b, in_=bout_d[0:1, :].partition_broadcast(128))
        nc.vector.tensor_copy(bias_c, bias_b)

        # ---- Phase 1: transpose x and context ----
        # convert to fp16 first: fp32 PE transposes run 4 cycles/row
        xh = persist.tile([128, NXT, D], F16, tag="xh", name="xh")
        ch = persist.tile([128, NST, D], F16, tag="ch", name="ch")
        idh = consts.tile([128, 128], F16, tag="idh", name="idh")
        nc.vector.tensor_copy(idh, identity)
        nc.vector.tensor_copy(xh, x_all)
        nc.vector.tensor_copy(ch, c_all)

        def do_transpose(src_all, st, j, dstT):
            pt = psum.tile([128, 128], F16, tag="sc", name="pt")
            nc.tensor.transpose(pt, src_all[:, st, 128 * j : 128 * j + 128], idh)
            nc.vector.tensor_copy(dstT[:, 128 * st : 128 * st + 128], pt)

        for t in range(NXT):
            for j in range(2):
                do_transpose(xh, t, j, xT[j])
        for st in range(NST):
            for j in range(2):
                do_transpose(ch, st, j, cT[j])

        # ---- Phase 2: projections ----
        for mj in range(2):
            for nt in range(TL // 512):
                pq = psum.tile([128, 512], FP, tag="sc", name="pq")
                for kj in range(2):
                    nc.tensor.matmul(
                        pq,
                        lhsT=wq[kj][:, 128 * mj : 128 * mj + 128],
                        rhs=xT[kj][:, 512 * nt : 512 * nt + 512],
                        start=(kj == 0),
                        stop=(kj == 1),
                    )
                for half in range(2):
                    nc.vector.tensor_copy(
                        qT[2 * mj + half][:, 512 * nt : 512 * nt + 512],
                        pq[64 * half : 64 * half + 64, :],
                    )
        for mj in range(2):
            for nt in range(S // 512):
                pk = psum.tile([128, 512], FP, tag="sc", name="pk")
                for kj in range(2):
                    nc.tensor.matmul(
                        pk,
                        lhsT=wkv[kj][:, 128 * mj : 128 * mj + 128],
                        rhs=cT[kj][:, 512 * nt : 512 * nt + 512],
                        start=(kj == 0),
                        stop=(kj == 1),
                    )
                for half in range(2):
                    nc.vector.tensor_copy(
                        kT[2 * mj + half][:, 512 * nt : 512 * nt + 512],
                        pk[64 * half : 64 * half + 64, :],
                    )
        for st in range(NST):
            pv = psum.tile([128, D], FP, tag="sc", name="pv")
            for kj in range(2):
                nc.tensor.matmul(
                    pv,
                    lhsT=cT[kj][:, 128 * st : 128 * st + 128],
                    rhs=wkv[kj][:, D : 2 * D],
                    start=(kj == 0),
                    stop=(kj == 1),
                )
            dst = vP[:, VW * st : VW * st + VW].rearrange("p (h w) -> p h w", h=H)[
                :, :, 0:Dh
            ]
            nc.vector.tensor_copy(dst, pv.rearrange("p (h w) -> p h w", h=H))
        ones_cols = vP.rearrange("p (s h w) -> p s h w", s=NST, h=H)[:, :, :, Dh : Dh + 1]
        nc.vector.memset(ones_cols, 1.0)

        # ---- Phase 3: fused attention ----
        # S-tiles processed in pairs: grouping the K=32 score matmuls and the
        # K=128 attn@v matmuls into runs halves the PE K-geometry switches
        # (~200ns each)
        dn_tiles = []
        for h in range(H):
            jj, aa = h // 2, h % 2
            tt_, bb_ = TRIP[h]
            acc = psum.tile([Dh + 1, TL], FP, tag="acc", name="acc", bufs=1)
            GRP = 4
            for sp in range(NST // GRP):
                sts = range(GRP * sp, GRP * sp + GRP)
                scs = []
                for st in sts:
                    sc = psum.tile([128, TL], FP, tag="sc", name="sc")
                    for nt in range(2):
                        nc.tensor.matmul(
                            sc[:, 512 * nt : 512 * nt + 512],
                            lhsT=kT[jj][32 * aa : 32 * aa + 32, 128 * st : 128 * st + 128],
                            rhs=qT[jj][32 * aa : 32 * aa + 32, 512 * nt : 512 * nt + 512],
                            start=True,
                            stop=True,
                            skip_group_check=True,
                        )
                    scs.append(sc)
                ats = []
                for sc in scs:
                    at = attnp.tile([128, TL], F16, tag="at", name="at")
                    nc.scalar.activation(
                        at, sc, mybir.ActivationFunctionType.Exp, scale=SCALE
                    )
                    ats.append(at)
                for i, st in enumerate(sts):
                    at = ats[i]
                    for nt in range(2):
                        nc.tensor.matmul(
                            acc[:, 512 * nt : 512 * nt + 512],
                            lhsT=vP[:, VW * st + (Dh + 1) * h : VW * st + (Dh + 1) * h + Dh + 1],
                            rhs=at[:, 512 * nt : 512 * nt + 512],
                            start=(st == 0),
                            stop=(st == NST - 1),
                            skip_group_check=True,
                        )
            nc.vector.tensor_copy(dumpT[tt_][32 * bb_ : 32 * bb_ + 32, :], acc[0:Dh, :])
            dnt = dnp.tile([1, TL], F16, tag="dn", name="dn")
            nc.vector.tensor_copy(dnt, acc[Dh : Dh + 1, :])
            dn_tiles.append(dnt)

        # ---- Phase 4: normalize + output projection ----
        for t in range(3):
            rp = psum.tile([96, TL], FP, tag="sc", name="rp")
            heads = TRIP_HEADS[t]
            for bi, h in enumerate(heads):
                for nt in range(2):
                    nc.tensor.matmul(
                        rp[:, 512 * nt : 512 * nt + 512],
                        lhsT=hsel[bi],
                        rhs=dn_tiles[h][:, 512 * nt : 512 * nt + 512],
                        start=(bi == 0),
                        stop=(bi == len(heads) - 1),
                        skip_group_check=True,
                    )
            # wide reciprocal of the broadcast denominators (a [1, TL]
            # single-partition reciprocal runs on one DVE lane ~50x slower)
            rps = rcp[t]
            with nc.allow_low_precision("softmax denominators are well-conditioned"):
                nc.vector.reciprocal(rps, rp)
            for bi in range(len(heads)):
                nc.vector.tensor_mul(
                    outTh[t][32 * bi : 32 * bi + 32, :],
                    dumpT[t][32 * bi : 32 * bi + 32, :],
                    rps[32 * bi : 32 * bi + 32, :],
                )
        for tt in range(TL // 128):
            fin = psum.tile([128, D], FP, tag="sc", name="fin")
            for t in range(3):
                nc.tensor.matmul(
                    fin,
                    lhsT=outTh[t][0 : wo_rows[t], 128 * tt : 128 * tt + 128],
                    rhs=woutg[t],
                    start=(t == 0),
                    stop=(t == 2),
                )
            outs = fstage.tile([128, D], FP, tag="outs", name="outs")
            nc.vector.tensor_add(outs, fin, bias_c)
            nc.sync.dma_start(out=out_d[128 * tt : 128 * tt + 128, :], in_=outs)

    nc.compile()
    return nc


_NC = None


def kernel(**inputs):
    global _NC
    x = np.ascontiguousarray(inputs["x"], dtype=np.float32)
    context = np.ascontiguousarray(inputs["context"], dtype=np.float32)
    w_q = np.ascontiguousarray(inputs["w_q"], dtype=np.float32)
    w_kv = np.ascontiguousarray(inputs["w_kv"], dtype=np.float32)
    w_out = np.ascontiguousarray(inputs["w_out"], dtype=np.float32)
    b_out = np.ascontiguousarray(inputs["b_out"], dtype=np.float32).reshape(1, D)

    if _NC is None:
        _NC = build_bass()
    nc = _NC

    in_maps = []
    for c in range(8):
        b, half = c // 2, c % 2
        in_maps.append(
            {
                "ident": np.eye(128, dtype=np.float32),
                "x": np.ascontiguousarray(x[b, TL * half : TL * half + TL, :]),
                "context": np.ascontiguousarray(context[b]),
                "w_q": w_q,
                "w_kv": w_kv,
                "w_out": w_out,
                "b_out": b_out,
            }
        )
    res = run_bass_kernel_spmd(nc, in_maps, core_ids=list(range(8)))
    out = np.empty((B, T, D), dtype=np.float32)
    for c in range(8):
        b, half = c // 2, c % 2
        out[b, TL * half : TL * half + TL, :] = res.results[c]["out"]
    return out


if __name__ == "__main__":
    rng = np.random.default_rng(0)
    ins = {
        "x": rng.standard_normal((B, T, D), dtype=np.float32),
        "context": rng.standard_normal((B, S, D), dtype=np.float32),
        "w_q": rng.standard_normal((D, D), dtype=np.float32) * D**-0.5,
        "w_kv": rng.standard_normal((D, 2 * D), dtype=np.float32) * D**-0.5,
        "w_out": rng.standard_normal((D, D), dtype=np.float32) * D**-0.5,
        "b_out": rng.standard_normal((D,), dtype=np.float32) * 0.01,
    }
    out = kernel(**ins)
    print(out.shape, out.dtype, np.abs(out).mean())
